# revision 22
# baseline (speedup 1.0000x reference)
"""Llama attention layer (B=1, S=2048, H=32, KVH=8, D=128, HID=4096) on 8 TRN2
NeuronCores.

Sharding: tensor-parallel over head groups. Core c computes Q heads
[4c..4c+4) and KV head c end-to-end (QKV projection, RoPE, causal GQA
attention, o_proj rows for its heads), then a chunked ReduceScatter sums the
o_proj partials so core c ends up with rows {512j + 64c .. 512j + 64c + 64}
of the output for j in 0..3. The host reassembles the full [2048, 4096]
output by concatenating the shards.

v1 design notes (vs the fp32r baseline):
  - All matmul operands are bf16: LDWEIGHTS drops to ~97ns and hides under
    the 216ns matmul issue rate (f32r weight loads are ~214ns and serialize).
  - All DMA'd tensors are bf16 (host converts): halves HBM and collective
    traffic. PSUM accumulation stays fp32.
  - QKV runs single-pass with all weights resident (6.1 MB bf16), chunk-
    outer so evictions/RoPE of chunk c overlap chunk c+1's matmuls.
  - RoPE rotate-half is a +-1 permutation matmul on the PE (rx = R @ x),
    replacing cross-partition SBUF DMAs; sign baked into R.
  - Causal diagonal tiles compute only the live column range [128j:512];
    only the first 128 columns of that range need the triangular mask.
  - Softmax denominators: ones-column matmul accumulates in PSUM; the
    reciprocal runs on a 128-partition broadcast via reciprocal_approx_fast
    (the [1,512] DVE reciprocal was 3.3us/tile).
  - o_proj partials stage through a [128, 4096] bf16 SBUF tile: one DMA per
    128-row band instead of 8.
  - ReduceScatter per 512-row chunk in bf16 overlaps later chunks' compute.
"""

import sys

if "/opt/trn_rl_repo" not in sys.path:
    sys.path.insert(0, "/opt/trn_rl_repo")

import numpy as np

# Model dims (hardcoded per problem spec)
H, KVH, D, HID = 32, 8, 128, 4096
S = 2048
THETA = 10000.0
NCORES = 8
QH = H // NCORES          # 4 query heads per core
P = 128                   # partitions
SC = 512                  # sequence chunk (matmul free dim)
NS = S // SC              # 4 chunks
KT = HID // P             # 32 contraction tiles for the projections
ST = S // P               # 16 sequence tiles of 128
NQK = QH + 2              # col-tiles per core in wqkv: q0..q3, k, v
WCOLS = NQK * P           # 768
ISQRT_D = float(D) ** -0.5
WGK = 4                   # weight k-tiles per DMA group
NWG = KT // WGK           # 8 weight groups
HGK = 16                  # hT k-tiles per DMA group
NHG = KT // HGK           # 2 hT groups per chunk

_CACHE = {}


def _build():
    import concourse.bass as bass
    import concourse.tile as tile
    from concourse import bacc, mybir
    from contextlib import ExitStack

    F32 = mybir.dt.float32
    BF16 = mybir.dt.bfloat16
    AF = mybir.ActivationFunctionType

    nc = bacc.Bacc(
        "TRN2",
        target_bir_lowering=False,
        debug=False,
        enable_asserts=False,
        num_devices=NCORES,
    )

    hT = nc.dram_tensor("hT", [HID, S], BF16, kind="ExternalInput").ap()
    wqkv = nc.dram_tensor("wqkv", [HID, WCOLS], BF16, kind="ExternalInput").ap()
    wo = nc.dram_tensor("wo", [QH * D, HID], BF16, kind="ExternalInput").ap()
    cos2 = nc.dram_tensor("cos2", [P, S], BF16, kind="ExternalInput").ap()
    sin2 = nc.dram_tensor("sin2", [P, S], BF16, kind="ExternalInput").ap()
    maskd = nc.dram_tensor("maskd", [P, P], BF16, kind="ExternalInput").ap()
    ident = nc.dram_tensor("ident", [P, P], BF16, kind="ExternalInput").ap()
    rperm = nc.dram_tensor("rperm", [P, P], BF16, kind="ExternalInput").ap()
    onesd = nc.dram_tensor("onesd", [P, 1], BF16, kind="ExternalInput").ap()
    out = nc.dram_tensor("out", [S // NCORES, HID], BF16, kind="ExternalOutput").ap()
    # per-chunk partial/rs tensors: a single DRAM tensor would WAR-serialize
    # chunk c's o_proj staging behind RS(c-1)'s reads (coarse DRAM dep
    # tracking)
    partial = [
        nc.dram_tensor(f"partial{c}", [SC, HID], BF16).ap() for c in range(NS)
    ]
    rs_out = nc.dram_tensor("rs_out", [S // NCORES, HID], BF16).ap()

    groups = [list(range(NCORES))]

    with tile.TileContext(nc) as tc:
        with ExitStack() as ctx:
            # ---------------- constants (whole-kernel lifetime) ----------------
            cpool = ctx.enter_context(tc.tile_pool(name="const", bufs=1))
            mask_t = cpool.tile([P, P], BF16, name="mask_t")
            ident_t = cpool.tile([P, P], BF16, name="ident_t")
            rperm_t = cpool.tile([P, P], BF16, name="rperm_t")
            ones_t = cpool.tile([P, 1], BF16, name="ones_t")
            nc.sync.dma_start(mask_t[:], maskd)
            nc.sync.dma_start(ident_t[:], ident)
            nc.sync.dma_start(rperm_t[:], rperm)
            nc.sync.dma_start(ones_t[:], onesd)

            # ------------- persistent activation buffers -------------
            ppool = ctx.enter_context(tc.tile_pool(name="persist", bufs=1))
            qk = {}
            for n in range(QH + 1):
                for c in range(NS):
                    qk[(n, c)] = ppool.tile(
                        [P, SC], BF16, name=f"qk{n}_{c}", tag=f"qk{n}_{c}"
                    )
            vT = [
                ppool.tile([P, SC], BF16, name=f"vT{c}", tag=f"vT{c}")
                for c in range(NS)
            ]
            vnat = [
                ppool.tile([P, P], BF16, name=f"vn{t}", tag=f"vn{t}")
                for t in range(ST)
            ]
            attnT = {}
            for h in range(QH):
                for c in range(NS):
                    attnT[(h, c)] = ppool.tile(
                        [P, SC], BF16, name=f"at{h}_{c}", tag=f"at{h}_{c}"
                    )

            # ---------------- stage A: QKV projection ----------------
            with tc.tile_pool(name="ropec", bufs=1) as rcpool, tc.tile_pool(
                name="wq", bufs=1
            ) as wq_pool, tc.tile_pool(name="ht", bufs=3) as h_pool, tc.tile_pool(
                name="psA", bufs=NQK, space="PSUM"
            ) as psA, tc.tile_pool(name="psR", bufs=1, space="PSUM") as psR, tc.tile_pool(
                name="ropet", bufs=4
            ) as rpool, tc.tile_pool(name="qsb", bufs=3) as qsb_pool:
                cos_t = rcpool.tile([P, S], BF16, name="cos_t")
                sin_t = rcpool.tile([P, S], BF16, name="sin_t")

                # resident QKV weights, 8 groups of 4 k-tiles; group 0 first
                # (first matmul needs it), the rest after cos/sin
                wq_g = []
                for g in range(NWG):
                    wt = wq_pool.tile([P, WGK * WCOLS], BF16, name=f"wqg{g}",
                                      tag=f"wq{g}")
                    src = wqkv[g * WGK * P : (g + 1) * WGK * P, :].rearrange(
                        "(t p) n -> p t n", p=P
                    )
                    nc.sync.dma_start(wt[:].rearrange("p (t n) -> p t n", t=WGK), src)
                    wq_g.append(wt)
                    if g == 0:
                        nc.sync.dma_start(cos_t[:], cos2)
                        nc.sync.dma_start(sin_t[:], sin2)

                def wslice(k, n):
                    g, kk = divmod(k, WGK)
                    off = kk * WCOLS + n * P
                    return wq_g[g][:, off : off + P]

                for c in range(NS):
                    # hT tiles for this chunk: 2 groups of 16 k-tiles.
                    # Loaded via the Act queue so the SP queue's weight loads
                    # don't serialize in front of them.
                    ht_g = []
                    for g in range(NHG):
                        ht = h_pool.tile([P, HGK * SC], BF16, name=f"ht{g}",
                                         tag="ht")
                        src = hT[
                            g * HGK * P : (g + 1) * HGK * P,
                            c * SC : (c + 1) * SC,
                        ].rearrange("(t p) n -> p t n", p=P)
                        nc.scalar.dma_start(
                            ht[:].rearrange("p (t n) -> p t n", t=HGK), src
                        )
                        ht_g.append(ht)

                    def htile(k):
                        g, kk = divmod(k, HGK)
                        return ht_g[g][:, kk * SC : (kk + 1) * SC]

                    ps = [
                        psA.tile([P, SC], F32, name=f"psA{n}", tag="psA")
                        for n in range(NQK)
                    ]
                    for k in range(KT):
                        rhs = htile(k)
                        for n in range(NQK):
                            nc.tensor.matmul(
                                ps[n][:], wslice(k, n), rhs,
                                start=(k == 0), stop=(k == KT - 1),
                            )

                    # evict + RoPE (q0..q3, k); v evicts straight to vT
                    csl = cos_t[:, c * SC : (c + 1) * SC]
                    ssl = sin_t[:, c * SC : (c + 1) * SC]
                    for n in range(QH + 1):
                        xsb = qsb_pool.tile([P, SC], BF16, name="xsb", tag="xsb")
                        nc.scalar.copy(xsb[:], ps[n][:])
                        # rx = R @ x  (half-swap with sign baked into R)
                        rx = psR.tile([P, SC], F32, name="rx", tag="rx")
                        nc.tensor.matmul(rx[:], rperm_t[:], xsb[:],
                                         start=True, stop=True)
                        t1 = rpool.tile([P, SC], BF16, name="t1", tag="t1")
                        t2 = rpool.tile([P, SC], BF16, name="t2", tag="t2")
                        nc.vector.tensor_mul(t1[:], xsb[:], csl)
                        nc.vector.tensor_mul(t2[:], rx[:], ssl)
                        nc.vector.tensor_add(qk[(n, c)][:], t1[:], t2[:])
                    nc.scalar.copy(vT[c][:], ps[NQK - 1][:])

                    # v transpose to natural [s, d] for this chunk
                    for j in range(NS):
                        t = c * NS + j
                        tp = psR.tile([P, P], BF16, name="tp", tag="tp")
                        nc.tensor.transpose(
                            tp[:], vT[c][:, j * P : (j + 1) * P], ident_t[:]
                        )
                        nc.scalar.copy(vnat[t][:], tp[:])

            # Wo resident [512, 4096] as 4 partition tiles
            wo_pool = ctx.enter_context(tc.tile_pool(name="wop", bufs=1))
            wo_t = [
                wo_pool.tile([P, HID], BF16, name=f"wo{hh}", tag=f"wo{hh}")
                for hh in range(QH)
            ]
            for hh in range(QH):
                nc.sync.dma_start(wo_t[hh][:], wo[hh * P : (hh + 1) * P, :])

            # ---------- attention + o_proj + reduce-scatter ----------
            with tc.tile_pool(name="pssc", bufs=2, space="PSUM") as ps_sc, tc.tile_pool(
                name="pssm", bufs=2, space="PSUM"
            ) as ps_sm, tc.tile_pool(
                name="pspv", bufs=2, space="PSUM"
            ) as ps_pv, tc.tile_pool(
                name="psop", bufs=2, space="PSUM"
            ) as ps_op, tc.tile_pool(name="expp", bufs=4) as ep, tc.tile_pool(
                name="smallp", bufs=2
            ) as sp, tc.tile_pool(name="stagep", bufs=4) as stp:
                for c in range(NS):
                    nsk = QH * c + QH  # causal: sk tiles for this chunk

                    # Flat (head, key-tile) sequence, software-pipelined one
                    # step: scores for step s+1 issue before sums/PV of step
                    # s, so the PE streams while the Act exp runs. Within a
                    # head, the diagonal tile j=0 goes first (it must own the
                    # full-width start=True write); the other diagonal tiles
                    # interleave with non-diagonal ones so their extra
                    # exp->mask latency hides behind clean tiles.
                    t_order = list(range(nsk))
                    steps = [(h, t, idx) for h in range(QH)
                             for idx, t in enumerate(t_order)]
                    pv_t = {}
                    sm_t = {}
                    e_t = {}

                    def emit_scores(s):
                        h, t, idx = steps[s]
                        kc, kj = divmod(t, NS)
                        diag = t - QH * c
                        lo = diag * P if diag >= 0 else 0
                        if idx == 0:
                            pv_t[h] = ps_pv.tile([P, SC], F32, name="pv",
                                                 tag="pv")
                            sm_t[h] = ps_sm.tile([1, SC], F32, name="sm",
                                                 tag="sm")
                        ktile = qk[(QH, kc)][:, kj * P : (kj + 1) * P]
                        sc_ps = ps_sc.tile([P, SC], F32, name="sc_ps", tag="sc")
                        nc.tensor.matmul(
                            sc_ps[:, lo:SC], ktile, qk[(h, c)][:, lo:SC],
                            start=True, stop=True,
                        )
                        e = ep.tile([P, SC], BF16, name="e", tag="e")
                        nc.scalar.activation(
                            e[:, lo:SC], sc_ps[:, lo:SC], AF.Exp, scale=ISQRT_D
                        )
                        if diag >= 0:
                            nc.vector.tensor_mul(
                                e[:, lo : lo + P], e[:, lo : lo + P], mask_t[:]
                            )
                        e_t[s] = (e, lo)

                    def emit_accum(s):
                        h, t, idx = steps[s]
                        e, lo = e_t.pop(s)
                        er = e[:, lo:SC]
                        nc.tensor.matmul(
                            sm_t[h][:, lo:SC], ones_t[:], er,
                            start=(idx == 0), stop=(idx == nsk - 1),
                            skip_group_check=True,
                        )
                        nc.tensor.matmul(
                            pv_t[h][:, lo:SC], vnat[t][:], er,
                            start=(idx == 0), stop=(idx == nsk - 1),
                            skip_group_check=True,
                        )
                        if idx == nsk - 1:
                            # normalize: broadcast the sum row via a K=1
                            # matmul (keeps gpsimd free for collectives)
                            smsb = sp.tile([1, SC], BF16, name="smsb",
                                           tag="smsb")
                            rc = sp.tile([P, SC], F32, name="rc", tag="rc")
                            nc.vector.tensor_copy(smsb[:], sm_t[h][:])
                            # broadcast the sum row to 128 partitions with a
                            # K=1 matmul; mask_t row 0 (triu) is all ones
                            bc = ps_sc.tile([P, SC], F32, name="bc", tag="sc")
                            nc.tensor.matmul(
                                bc[:], mask_t[0:1, :], smsb[:],
                                start=True, stop=True,
                            )
                            nc.vector.reciprocal_approx_fast(rc[:], bc[:])
                            nc.vector.tensor_mul(
                                attnT[(h, c)][:], pv_t.pop(h)[:], rc[:]
                            )
                            sm_t.pop(h)

                    emit_scores(0)
                    for s in range(1, len(steps)):
                        emit_scores(s)
                        emit_accum(s - 1)
                    emit_accum(len(steps) - 1)

                    # o_proj for the 4 sequence tiles of this chunk
                    for i in range(QH * c, QH * c + QH):
                        jj = i - QH * c
                        st = stp.tile([P, HID], BF16, name="st", tag="st")
                        for nn in range(HID // SC):
                            op = ps_op.tile([P, SC], F32, name="op", tag="op")
                            for h in range(QH):
                                nc.tensor.matmul(
                                    op[:],
                                    attnT[(h, c)][:, jj * P : (jj + 1) * P],
                                    wo_t[h][:, nn * SC : (nn + 1) * SC],
                                    start=(h == 0),
                                    stop=(h == QH - 1),
                                )
                            nc.scalar.copy(st[:, nn * SC : (nn + 1) * SC], op[:])
                        nc.sync.dma_start(partial[c][jj * P : (jj + 1) * P, :], st[:])

                        if c == NS - 1 and jj % 2 == 1:
                            half = jj // 2
                            nc.gpsimd.collective_compute(
                                "ReduceScatter",
                                mybir.AluOpType.add,
                                replica_groups=groups,
                                ins=[partial[c][half * 2 * P :
                                               (half + 1) * 2 * P, :]],
                                outs=[rs_out[c * 64 + half * 32 :
                                             c * 64 + (half + 1) * 32, :]],
                            )

                    # reduce-scatter this chunk across the 8 cores; the
                    # last chunk goes in halves so the first half overlaps
                    # the second half's o_proj
                    if c < NS - 1:
                        nc.gpsimd.collective_compute(
                            "ReduceScatter",
                            mybir.AluOpType.add,
                            replica_groups=groups,
                            ins=[partial[c][:, :]],
                            outs=[rs_out[c * 64 : (c + 1) * 64, :]],
                        )

                # rs_out -> out copies deferred to the end so their DMA
                # completions (gated on the collectives) never sit in front
                # of mid-kernel partial writes in a completion ring.
                nc.gpsimd.dma_start(out[:, :], rs_out[:, :])

    nc.compile()
    return nc


def _get_nc():
    if "nc" not in _CACHE:
        _CACHE["nc"] = _build()
    return _CACHE["nc"]


def _host_inputs(positions, hidden_states, Wqkv, Wo):
    """Shard + relayout the full inputs for the 8 cores (bf16)."""
    import ml_dtypes

    BF = ml_dtypes.bfloat16
    pos = np.asarray(positions).reshape(-1).astype(np.float32)  # [S]
    hs = np.asarray(hidden_states, dtype=np.float32).reshape(S, HID)
    Wqkv = np.asarray(Wqkv, dtype=np.float32)
    Wo = np.asarray(Wo, dtype=np.float32)

    hT = np.ascontiguousarray(hs.T).astype(BF)  # [HID, S]

    half = D // 2
    inv_freq = 1.0 / (THETA ** (np.arange(half, dtype=np.float32) / half))
    ang = pos[None, :] * inv_freq[:, None]  # [64, S]
    cos = np.cos(ang).astype(np.float32)
    sin = np.sin(ang).astype(np.float32)
    cos2 = np.concatenate([cos, cos], axis=0).astype(BF)  # [128, S]
    sin2 = np.concatenate([sin, sin], axis=0).astype(BF)  # [128, S]

    maskd = np.triu(np.ones((P, P), dtype=np.float32)).astype(BF)
    ident = np.eye(P, dtype=np.float32).astype(BF)
    onesd = np.ones((P, 1), dtype=np.float32).astype(BF)

    # rotate-half permutation with sign baked in: (R @ x)[m] =
    #   -x[m+64] for m < 64, +x[m-64] for m >= 64.
    # matmul computes lhsT.T @ rhs, so lhsT[p, m] = R[m, p].
    rp = np.zeros((P, P), dtype=np.float32)
    for m in range(half):
        rp[m + half, m] = -1.0
    for m in range(half, P):
        rp[m - half, m] = 1.0
    rperm = rp.astype(BF)

    qb = Wqkv[:, : H * D]
    kb = Wqkv[:, H * D : H * D + KVH * D]
    vb = Wqkv[:, H * D + KVH * D :]

    in_maps = []
    for c in range(NCORES):
        wq_c = np.concatenate(
            [
                qb[:, c * QH * D : (c + 1) * QH * D],
                kb[:, c * D : (c + 1) * D],
                vb[:, c * D : (c + 1) * D],
            ],
            axis=1,
        )
        wo_c = Wo[c * QH * D : (c + 1) * QH * D, :]
        in_maps.append(
            {
                "hT": hT,
                "wqkv": np.ascontiguousarray(wq_c).astype(BF),
                "wo": np.ascontiguousarray(wo_c).astype(BF),
                "cos2": cos2,
                "sin2": sin2,
                "maskd": maskd,
                "ident": ident,
                "rperm": rperm,
                "onesd": onesd,
            }
        )
    return in_maps


def _assemble(results):
    full = np.empty((S, HID), dtype=np.float32)
    for c in range(NCORES):
        oc = np.asarray(results[c]["out"], dtype=np.float32)  # [256, HID]
        for j in range(NS - 1):
            full[SC * j + 64 * c : SC * j + 64 * (c + 1), :] = oc[
                64 * j : 64 * (j + 1), :
            ]
        # chunk 3 ran as two half reduce-scatters: half i of core c holds
        # global rows 1536 + 256*i + 32*c .. +32
        for i in range(2):
            lo = SC * (NS - 1) + 256 * i + 32 * c
            full[lo : lo + 32, :] = oc[192 + 32 * i : 192 + 32 * (i + 1), :]
    return full.reshape(1, S, HID)


def kernel(positions, hidden_states, Wqkv, Wo):
    from concourse.bass_utils import run_bass_kernel_spmd

    nc = _get_nc()
    in_maps = _host_inputs(positions, hidden_states, Wqkv, Wo)
    res = run_bass_kernel_spmd(nc, in_maps, core_ids=list(range(NCORES)))
    return _assemble(res.results)


def kernel_timed(positions, hidden_states, Wqkv, Wo):
    """Like kernel() but with NTFF profiling; returns (output, exec_time_ns)."""
    from concourse.bass_utils import run_bass_kernel_spmd

    nc = _get_nc()
    in_maps = _host_inputs(positions, hidden_states, Wqkv, Wo)
    res = run_bass_kernel_spmd(
        nc, in_maps, core_ids=list(range(NCORES)), trace=True
    )
    return _assemble(res.results), res.exec_time_ns


# revision 23
# speedup vs baseline: 1.2190x; 1.2190x over previous
"""Llama attention layer (B=1, S=2048, H=32, KVH=8, D=128, HID=4096) on 8 TRN2
NeuronCores.

Sharding: tensor-parallel over head groups. Core c computes Q heads
[4c..4c+4) and KV head c end-to-end (QKV projection, RoPE, causal GQA
attention, o_proj rows for its heads), then a chunked ReduceScatter sums the
o_proj partials so core c ends up with rows {512j + 64c .. 512j + 64c + 64}
of the output for j in 0..3. The host reassembles the full [2048, 4096]
output by concatenating the shards.

v1 design notes (vs the fp32r baseline):
  - All matmul operands are bf16: LDWEIGHTS drops to ~97ns and hides under
    the 216ns matmul issue rate (f32r weight loads are ~214ns and serialize).
  - All DMA'd tensors are bf16 (host converts): halves HBM and collective
    traffic. PSUM accumulation stays fp32.
  - QKV runs single-pass with all weights resident (6.1 MB bf16), chunk-
    outer so evictions/RoPE of chunk c overlap chunk c+1's matmuls.
  - RoPE rotate-half is a +-1 permutation matmul on the PE (rx = R @ x),
    replacing cross-partition SBUF DMAs; sign baked into R.
  - Causal diagonal tiles compute only the live column range [128j:512];
    only the first 128 columns of that range need the triangular mask.
  - Softmax denominators: ones-column matmul accumulates in PSUM; the
    reciprocal runs on a 128-partition broadcast via reciprocal_approx_fast
    (the [1,512] DVE reciprocal was 3.3us/tile).
  - o_proj partials stage through a [128, 4096] bf16 SBUF tile: one DMA per
    128-row band instead of 8.
  - ReduceScatter per 512-row chunk in bf16 overlaps later chunks' compute.
"""

import sys

if "/opt/trn_rl_repo" not in sys.path:
    sys.path.insert(0, "/opt/trn_rl_repo")

import numpy as np

# Model dims (hardcoded per problem spec)
H, KVH, D, HID = 32, 8, 128, 4096
S = 2048
THETA = 10000.0
NCORES = 8
QH = H // NCORES          # 4 query heads per core
P = 128                   # partitions
SC = 512                  # sequence chunk (matmul free dim)
NS = S // SC              # 4 chunks
KT = HID // P             # 32 contraction tiles for the projections
ST = S // P               # 16 sequence tiles of 128
NQK = QH + 2              # col-tiles per core in wqkv: q0..q3, k, v
WCOLS = NQK * P           # 768
ISQRT_D = float(D) ** -0.5
WGK = 4                   # weight k-tiles per DMA group
NWG = KT // WGK           # 8 weight groups
HGK = 16                  # hT k-tiles per DMA group
NHG = KT // HGK           # 2 hT groups per chunk

_CACHE = {}


def _build():
    import concourse.bass as bass
    import concourse.tile as tile
    from concourse import bacc, mybir
    from contextlib import ExitStack

    F32 = mybir.dt.float32
    BF16 = mybir.dt.bfloat16
    AF = mybir.ActivationFunctionType

    nc = bacc.Bacc(
        "TRN2",
        target_bir_lowering=False,
        debug=False,
        enable_asserts=False,
        num_devices=NCORES,
    )

    hT = nc.dram_tensor("hT", [HID, S], BF16, kind="ExternalInput").ap()
    wqkv = nc.dram_tensor("wqkv", [HID, WCOLS], BF16, kind="ExternalInput").ap()
    wo = nc.dram_tensor("wo", [QH * D, HID], BF16, kind="ExternalInput").ap()
    cos2 = nc.dram_tensor("cos2", [P, S], BF16, kind="ExternalInput").ap()
    sin2 = nc.dram_tensor("sin2", [P, S], BF16, kind="ExternalInput").ap()
    maskd = nc.dram_tensor("maskd", [P, P], BF16, kind="ExternalInput").ap()
    ident = nc.dram_tensor("ident", [P, P], BF16, kind="ExternalInput").ap()
    rperm = nc.dram_tensor("rperm", [P, P], BF16, kind="ExternalInput").ap()
    onesd = nc.dram_tensor("onesd", [P, 1], BF16, kind="ExternalInput").ap()
    out = nc.dram_tensor("out", [S // NCORES, HID], BF16, kind="ExternalOutput").ap()
    # per-chunk partial/rs tensors: a single DRAM tensor would WAR-serialize
    # chunk c's o_proj staging behind RS(c-1)'s reads (coarse DRAM dep
    # tracking)
    partial = [
        nc.dram_tensor(f"partial{c}", [SC, HID], BF16).ap() for c in range(NS)
    ]
    # one rs_out per collective: shared output would WAW-serialize the
    # collectives and stop the CC firmware from pipelining them
    rs_out = [
        nc.dram_tensor(f"rs_out{c}", [SC // NCORES, HID], BF16).ap()
        for c in range(NS - 1)
    ] + [
        nc.dram_tensor(f"rs_out3{h}", [SC // (2 * NCORES), HID], BF16).ap()
        for h in range(2)
    ]

    groups = [list(range(NCORES))]

    with tile.TileContext(nc) as tc:
        with ExitStack() as ctx:
            # ---------------- constants (whole-kernel lifetime) ----------------
            cpool = ctx.enter_context(tc.tile_pool(name="const", bufs=1))
            mask_t = cpool.tile([P, P], BF16, name="mask_t")
            ident_t = cpool.tile([P, P], BF16, name="ident_t")
            rperm_t = cpool.tile([P, P], BF16, name="rperm_t")
            ones_t = cpool.tile([P, 1], BF16, name="ones_t")
            nc.sync.dma_start(mask_t[:], maskd)
            nc.sync.dma_start(ident_t[:], ident)
            nc.sync.dma_start(rperm_t[:], rperm)
            nc.sync.dma_start(ones_t[:], onesd)

            # ------------- persistent activation buffers -------------
            ppool = ctx.enter_context(tc.tile_pool(name="persist", bufs=1))
            qk = {}
            for n in range(QH + 1):
                for c in range(NS):
                    qk[(n, c)] = ppool.tile(
                        [P, SC], BF16, name=f"qk{n}_{c}", tag=f"qk{n}_{c}"
                    )
            vT = [
                ppool.tile([P, SC], BF16, name=f"vT{c}", tag=f"vT{c}")
                for c in range(NS)
            ]
            vnat = [
                ppool.tile([P, P], BF16, name=f"vn{t}", tag=f"vn{t}")
                for t in range(ST)
            ]
            attnT = {}
            for h in range(QH):
                for c in range(NS):
                    attnT[(h, c)] = ppool.tile(
                        [P, SC], BF16, name=f"at{h}_{c}", tag=f"at{h}_{c}"
                    )

            # ---------------- stage A: QKV projection ----------------
            with tc.tile_pool(name="ropec", bufs=1) as rcpool, tc.tile_pool(
                name="wq", bufs=1
            ) as wq_pool, tc.tile_pool(name="ht", bufs=3) as h_pool, tc.tile_pool(
                name="psA", bufs=NQK, space="PSUM"
            ) as psA, tc.tile_pool(name="psR", bufs=1, space="PSUM") as psR, tc.tile_pool(
                name="ropet", bufs=4
            ) as rpool, tc.tile_pool(name="qsb", bufs=3) as qsb_pool:
                cos_t = rcpool.tile([P, S], BF16, name="cos_t")
                sin_t = rcpool.tile([P, S], BF16, name="sin_t")

                # resident QKV weights, 8 groups of 4 k-tiles; group 0 first
                # (first matmul needs it), the rest after cos/sin
                wq_g = []
                for g in range(NWG):
                    wt = wq_pool.tile([P, WGK * WCOLS], BF16, name=f"wqg{g}",
                                      tag=f"wq{g}")
                    src = wqkv[g * WGK * P : (g + 1) * WGK * P, :].rearrange(
                        "(t p) n -> p t n", p=P
                    )
                    nc.sync.dma_start(wt[:].rearrange("p (t n) -> p t n", t=WGK), src)
                    wq_g.append(wt)
                    if g == 0:
                        nc.sync.dma_start(cos_t[:], cos2)
                        nc.sync.dma_start(sin_t[:], sin2)

                def wslice(k, n):
                    g, kk = divmod(k, WGK)
                    off = kk * WCOLS + n * P
                    return wq_g[g][:, off : off + P]

                for c in range(NS):
                    # hT tiles for this chunk: 2 groups of 16 k-tiles.
                    # Loaded via the Act queue so the SP queue's weight loads
                    # don't serialize in front of them.
                    ht_g = []
                    for g in range(NHG):
                        ht = h_pool.tile([P, HGK * SC], BF16, name=f"ht{g}",
                                         tag="ht")
                        src = hT[
                            g * HGK * P : (g + 1) * HGK * P,
                            c * SC : (c + 1) * SC,
                        ].rearrange("(t p) n -> p t n", p=P)
                        nc.scalar.dma_start(
                            ht[:].rearrange("p (t n) -> p t n", t=HGK), src
                        )
                        ht_g.append(ht)

                    def htile(k):
                        g, kk = divmod(k, HGK)
                        return ht_g[g][:, kk * SC : (kk + 1) * SC]

                    ps = [
                        psA.tile([P, SC], F32, name=f"psA{n}", tag="psA")
                        for n in range(NQK)
                    ]
                    for k in range(KT):
                        rhs = htile(k)
                        for n in range(NQK):
                            nc.tensor.matmul(
                                ps[n][:], wslice(k, n), rhs,
                                start=(k == 0), stop=(k == KT - 1),
                            )

                    # evict + RoPE (q0..q3, k); v evicts straight to vT
                    csl = cos_t[:, c * SC : (c + 1) * SC]
                    ssl = sin_t[:, c * SC : (c + 1) * SC]
                    for n in range(QH + 1):
                        xsb = qsb_pool.tile([P, SC], BF16, name="xsb", tag="xsb")
                        nc.scalar.copy(xsb[:], ps[n][:])
                        # rx = R @ x  (half-swap with sign baked into R)
                        rx = psR.tile([P, SC], F32, name="rx", tag="rx")
                        nc.tensor.matmul(rx[:], rperm_t[:], xsb[:],
                                         start=True, stop=True)
                        t1 = rpool.tile([P, SC], BF16, name="t1", tag="t1")
                        t2 = rpool.tile([P, SC], BF16, name="t2", tag="t2")
                        nc.vector.tensor_mul(t1[:], xsb[:], csl)
                        nc.vector.tensor_mul(t2[:], rx[:], ssl)
                        nc.vector.tensor_add(qk[(n, c)][:], t1[:], t2[:])
                    nc.scalar.copy(vT[c][:], ps[NQK - 1][:])

                    # v transpose to natural [s, d] for this chunk
                    for j in range(NS):
                        t = c * NS + j
                        tp = psR.tile([P, P], BF16, name="tp", tag="tp")
                        nc.tensor.transpose(
                            tp[:], vT[c][:, j * P : (j + 1) * P], ident_t[:]
                        )
                        nc.scalar.copy(vnat[t][:], tp[:])

            # Wo resident [512, 4096] as 4 partition tiles
            wo_pool = ctx.enter_context(tc.tile_pool(name="wop", bufs=1))
            wo_t = [
                wo_pool.tile([P, HID], BF16, name=f"wo{hh}", tag=f"wo{hh}")
                for hh in range(QH)
            ]
            for hh in range(QH):
                nc.sync.dma_start(wo_t[hh][:], wo[hh * P : (hh + 1) * P, :])

            # ---------- attention + o_proj + reduce-scatter ----------
            with tc.tile_pool(name="pssc", bufs=2, space="PSUM") as ps_sc, tc.tile_pool(
                name="pssm", bufs=2, space="PSUM"
            ) as ps_sm, tc.tile_pool(
                name="pspv", bufs=2, space="PSUM"
            ) as ps_pv, tc.tile_pool(
                name="psop", bufs=2, space="PSUM"
            ) as ps_op, tc.tile_pool(name="expp", bufs=4) as ep, tc.tile_pool(
                name="smallp", bufs=2
            ) as sp, tc.tile_pool(name="stagep", bufs=4) as stp:
                for c in range(NS):
                    nsk = QH * c + QH  # causal: sk tiles for this chunk

                    # Flat (head, key-tile) sequence, software-pipelined one
                    # step: scores for step s+1 issue before sums/PV of step
                    # s, so the PE streams while the Act exp runs. Within a
                    # head, the diagonal tile j=0 goes first (it must own the
                    # full-width start=True write); the other diagonal tiles
                    # interleave with non-diagonal ones so their extra
                    # exp->mask latency hides behind clean tiles.
                    t_order = list(range(nsk))
                    steps = [(h, t, idx) for h in range(QH)
                             for idx, t in enumerate(t_order)]
                    pv_t = {}
                    sm_t = {}
                    e_t = {}

                    def emit_scores(s):
                        h, t, idx = steps[s]
                        kc, kj = divmod(t, NS)
                        diag = t - QH * c
                        lo = diag * P if diag >= 0 else 0
                        if idx == 0:
                            pv_t[h] = ps_pv.tile([P, SC], F32, name="pv",
                                                 tag="pv")
                            sm_t[h] = ps_sm.tile([1, SC], F32, name="sm",
                                                 tag="sm")
                        ktile = qk[(QH, kc)][:, kj * P : (kj + 1) * P]
                        sc_ps = ps_sc.tile([P, SC], F32, name="sc_ps", tag="sc")
                        nc.tensor.matmul(
                            sc_ps[:, lo:SC], ktile, qk[(h, c)][:, lo:SC],
                            start=True, stop=True,
                        )
                        e = ep.tile([P, SC], BF16, name="e", tag="e")
                        nc.scalar.activation(
                            e[:, lo:SC], sc_ps[:, lo:SC], AF.Exp, scale=ISQRT_D
                        )
                        if diag >= 0:
                            nc.vector.tensor_mul(
                                e[:, lo : lo + P], e[:, lo : lo + P], mask_t[:]
                            )
                        e_t[s] = (e, lo)

                    def emit_accum(s):
                        h, t, idx = steps[s]
                        e, lo = e_t.pop(s)
                        er = e[:, lo:SC]
                        nc.tensor.matmul(
                            sm_t[h][:, lo:SC], ones_t[:], er,
                            start=(idx == 0), stop=(idx == nsk - 1),
                            skip_group_check=True,
                        )
                        nc.tensor.matmul(
                            pv_t[h][:, lo:SC], vnat[t][:], er,
                            start=(idx == 0), stop=(idx == nsk - 1),
                            skip_group_check=True,
                        )
                        if idx == nsk - 1:
                            # normalize: broadcast the sum row via a K=1
                            # matmul (keeps gpsimd free for collectives)
                            smsb = sp.tile([1, SC], BF16, name="smsb",
                                           tag="smsb")
                            rc = sp.tile([P, SC], F32, name="rc", tag="rc")
                            nc.vector.tensor_copy(smsb[:], sm_t[h][:])
                            # broadcast the sum row to 128 partitions with a
                            # K=1 matmul; mask_t row 0 (triu) is all ones
                            bc = ps_sc.tile([P, SC], F32, name="bc", tag="sc")
                            nc.tensor.matmul(
                                bc[:], mask_t[0:1, :], smsb[:],
                                start=True, stop=True,
                            )
                            nc.vector.reciprocal_approx_fast(rc[:], bc[:])
                            nc.vector.tensor_mul(
                                attnT[(h, c)][:], pv_t.pop(h)[:], rc[:]
                            )
                            sm_t.pop(h)

                    emit_scores(0)
                    for s in range(1, len(steps)):
                        emit_scores(s)
                        emit_accum(s - 1)
                    emit_accum(len(steps) - 1)

                    # o_proj for the 4 sequence tiles of this chunk
                    for i in range(QH * c, QH * c + QH):
                        jj = i - QH * c
                        st = stp.tile([P, HID], BF16, name="st", tag="st")
                        for nn in range(HID // SC):
                            op = ps_op.tile([P, SC], F32, name="op", tag="op")
                            for h in range(QH):
                                nc.tensor.matmul(
                                    op[:],
                                    attnT[(h, c)][:, jj * P : (jj + 1) * P],
                                    wo_t[h][:, nn * SC : (nn + 1) * SC],
                                    start=(h == 0),
                                    stop=(h == QH - 1),
                                )
                            nc.scalar.copy(st[:, nn * SC : (nn + 1) * SC], op[:])
                        nc.sync.dma_start(partial[c][jj * P : (jj + 1) * P, :], st[:])

                        if c == NS - 1 and jj % 2 == 1:
                            half = jj // 2
                            nc.gpsimd.collective_compute(
                                "ReduceScatter",
                                mybir.AluOpType.add,
                                replica_groups=groups,
                                ins=[partial[c][half * 2 * P :
                                               (half + 1) * 2 * P, :]],
                                outs=[rs_out[NS - 1 + half][:, :]],
                            )

                    # reduce-scatter this chunk across the 8 cores; the
                    # last chunk goes in halves so the first half overlaps
                    # the second half's o_proj
                    if c < NS - 1:
                        nc.gpsimd.collective_compute(
                            "ReduceScatter",
                            mybir.AluOpType.add,
                            replica_groups=groups,
                            ins=[partial[c][:, :]],
                            outs=[rs_out[c][:, :]],
                        )

                # rs_out -> out copies deferred to the end so their DMA
                # completions (gated on the collectives) never sit in front
                # of mid-kernel partial writes in a completion ring.
                for c in range(NS - 1):
                    nc.gpsimd.dma_start(
                        out[c * 64 : (c + 1) * 64, :], rs_out[c][:, :]
                    )
                for h in range(2):
                    nc.gpsimd.dma_start(
                        out[192 + h * 32 : 192 + (h + 1) * 32, :],
                        rs_out[NS - 1 + h][:, :],
                    )

    nc.compile()
    return nc


def _get_nc():
    if "nc" not in _CACHE:
        _CACHE["nc"] = _build()
    return _CACHE["nc"]


def _host_inputs(positions, hidden_states, Wqkv, Wo):
    """Shard + relayout the full inputs for the 8 cores (bf16)."""
    import ml_dtypes

    BF = ml_dtypes.bfloat16
    pos = np.asarray(positions).reshape(-1).astype(np.float32)  # [S]
    hs = np.asarray(hidden_states, dtype=np.float32).reshape(S, HID)
    Wqkv = np.asarray(Wqkv, dtype=np.float32)
    Wo = np.asarray(Wo, dtype=np.float32)

    hT = np.ascontiguousarray(hs.T).astype(BF)  # [HID, S]

    half = D // 2
    inv_freq = 1.0 / (THETA ** (np.arange(half, dtype=np.float32) / half))
    ang = pos[None, :] * inv_freq[:, None]  # [64, S]
    cos = np.cos(ang).astype(np.float32)
    sin = np.sin(ang).astype(np.float32)
    cos2 = np.concatenate([cos, cos], axis=0).astype(BF)  # [128, S]
    sin2 = np.concatenate([sin, sin], axis=0).astype(BF)  # [128, S]

    maskd = np.triu(np.ones((P, P), dtype=np.float32)).astype(BF)
    ident = np.eye(P, dtype=np.float32).astype(BF)
    onesd = np.ones((P, 1), dtype=np.float32).astype(BF)

    # rotate-half permutation with sign baked in: (R @ x)[m] =
    #   -x[m+64] for m < 64, +x[m-64] for m >= 64.
    # matmul computes lhsT.T @ rhs, so lhsT[p, m] = R[m, p].
    rp = np.zeros((P, P), dtype=np.float32)
    for m in range(half):
        rp[m + half, m] = -1.0
    for m in range(half, P):
        rp[m - half, m] = 1.0
    rperm = rp.astype(BF)

    qb = Wqkv[:, : H * D]
    kb = Wqkv[:, H * D : H * D + KVH * D]
    vb = Wqkv[:, H * D + KVH * D :]

    in_maps = []
    for c in range(NCORES):
        wq_c = np.concatenate(
            [
                qb[:, c * QH * D : (c + 1) * QH * D],
                kb[:, c * D : (c + 1) * D],
                vb[:, c * D : (c + 1) * D],
            ],
            axis=1,
        )
        wo_c = Wo[c * QH * D : (c + 1) * QH * D, :]
        in_maps.append(
            {
                "hT": hT,
                "wqkv": np.ascontiguousarray(wq_c).astype(BF),
                "wo": np.ascontiguousarray(wo_c).astype(BF),
                "cos2": cos2,
                "sin2": sin2,
                "maskd": maskd,
                "ident": ident,
                "rperm": rperm,
                "onesd": onesd,
            }
        )
    return in_maps


def _assemble(results):
    full = np.empty((S, HID), dtype=np.float32)
    for c in range(NCORES):
        oc = np.asarray(results[c]["out"], dtype=np.float32)  # [256, HID]
        for j in range(NS - 1):
            full[SC * j + 64 * c : SC * j + 64 * (c + 1), :] = oc[
                64 * j : 64 * (j + 1), :
            ]
        # chunk 3 ran as two half reduce-scatters: half i of core c holds
        # global rows 1536 + 256*i + 32*c .. +32
        for i in range(2):
            lo = SC * (NS - 1) + 256 * i + 32 * c
            full[lo : lo + 32, :] = oc[192 + 32 * i : 192 + 32 * (i + 1), :]
    return full.reshape(1, S, HID)


def kernel(positions, hidden_states, Wqkv, Wo):
    from concourse.bass_utils import run_bass_kernel_spmd

    nc = _get_nc()
    in_maps = _host_inputs(positions, hidden_states, Wqkv, Wo)
    res = run_bass_kernel_spmd(nc, in_maps, core_ids=list(range(NCORES)))
    return _assemble(res.results)


def kernel_timed(positions, hidden_states, Wqkv, Wo):
    """Like kernel() but with NTFF profiling; returns (output, exec_time_ns)."""
    from concourse.bass_utils import run_bass_kernel_spmd

    nc = _get_nc()
    in_maps = _host_inputs(positions, hidden_states, Wqkv, Wo)
    res = run_bass_kernel_spmd(
        nc, in_maps, core_ids=list(range(NCORES)), trace=True
    )
    return _assemble(res.results), res.exec_time_ns
